# revision 16
# baseline (speedup 1.0000x reference)
"""Trainium2 Bass kernel for nn_Concat_Linear (feat [65536,2,768] -> out [65536,9]).

Data-parallel across 8 NeuronCores (8192 rows each). Per core, fp32 throughout:

  - feat rows are DMA'd FULLY CONTIGUOUSLY (near-HBM-rate): partition q holds
    rows 8q..8q+7 of the buffer.
  - the contraction-on-partitions layout is produced ON-CHIP by DVE
    StreamTranspose (32x32 blocks) into TR[32g+f, bj, col] = feat[row, 32bj+f]
    with col = 128*(u&1) + 4c + (u>>1) for row = 256g + 8c + u, a column order
    chosen so the final store comes out in 72-byte contiguous runs; the
    projection then runs as 4 concurrent K=32 accumulation chains
    (tile_position=(32g,0)) x 48 bj-steps with N=256 contiguous moving data.
  - epilogue (trilinear form, LayerNorm, final linear) runs feature-on-partition
    via small PE matmuls; elementwise work is spread over DVE/ACT/GPSIMD:
    the LN variance is a gpsimd partition_all_reduce (which also broadcasts),
    rstd = exp(-0.5*ln(var/9+eps)) on ACT, multiplies on gpsimd.
  - outputs are PE-transposed back to row-major and stored per 512 rows.
"""

import sys
import types

import numpy as np

B_FULL = 65536
N_CORES = 8
B_CORE = B_FULL // N_CORES
D = 1536       # 2 * 768
NB = 1024      # rows per buffer
NCHUNK = 4     # d-chunks per buffer (load DMA + transpose granularity)
DC = D // NCHUNK          # 384 features per chunk
BJC = DC // 32            # 12 feature-blocks per chunk
LN_EPS = 1e-5

import os
# variant switches (bisection / perf A-B)
HOST_BF16 = os.environ.get("K_HOST_BF16", "0") == "1"  # host casts feat, HWDGE load
VAR_PE = os.environ.get("K_VAR_PE", "0") == "1"        # PE matmul var path
                                                       # instead of gpsimd allreduce


def _ensure_axon_hooks():
    """Register the NTFF profile hook if the image's antenv lacks axon_hooks.

    Without this, trace=True degrades to no profiling (runs still work)."""
    try:
        import antenv  # noqa: F401
        from antenv import axon_hooks  # noqa: F401
        return
    except ImportError:
        pass
    try:
        import antenv
        mod = types.ModuleType("antenv.axon_hooks")
        mod._hook = None
        mod.set_axon_ntff_profile_hook = lambda h: setattr(mod, "_hook", h)
        mod.get_axon_ntff_profile_hook = lambda: mod._hook
        sys.modules["antenv.axon_hooks"] = mod
        antenv.axon_hooks = mod
        from trn_agent_boot.trn_boot import _ntff_profile_via_ctypes
        mod.set_axon_ntff_profile_hook(
            _ntff_profile_via_ctypes("/opt/axon/libaxon_pjrt.so")
        )
    except Exception:
        pass


def make_consts(W_int, W_stim, trans, ln_w, ln_b, W_out, b_out):
    """Host-side constant tensors (all fp32)."""
    W_int = np.asarray(W_int, np.float32)
    W_stim = np.asarray(W_stim, np.float32)
    trans = np.asarray(trans, np.float32)
    ln_w = np.asarray(ln_w, np.float32)
    ln_b = np.asarray(ln_b, np.float32)
    W_out = np.asarray(W_out, np.float32)
    b_out = np.asarray(b_out, np.float32)

    # Projection: Y[0:9]  = this = feat[:,1,:] @ W_stim.T   (d in [768,1536))
    #             Y[32:41] = last = feat[:,0,:] @ W_int.T   (d in [0,768))
    import ml_dtypes
    W_cat = np.zeros((D, 41), np.float32)
    W_cat[768:1536, 0:9] = W_stim.T
    W_cat[0:768, 32:41] = W_int.T
    # ws[32g+f, bj, :] = W_cat[32bj+f, :]  (replicated over the 4 row slabs g)
    ws = np.zeros((128, 48, 41), np.float32)
    for g in range(4):
        for f in range(32):
            ws[32 * g + f, :, :] = W_cat[f::32, :].reshape(48, 41)
    ws = ws.astype(ml_dtypes.bfloat16)

    # trans matrix for G[a*9+k, b] = sum_j trans[a,j,k] * last[j, b]
    # rows live at partitions 32:41 to match last's position in Y.
    tm = np.zeros((41, 81), np.float32)
    for a in range(9):
        for j in range(9):
            for k in range(9):
                tm[32 + j, a * 9 + k] = trans[a, j, k]

    # thisbc[a*9+k, b] = this[a, b]
    e9 = np.zeros((9, 81), np.float32)
    for a in range(9):
        e9[a, a * 9:(a + 1) * 9] = 1.0

    # bil_centered[k', b] = sum_a M[a*9+k', b] - (1/9) sum_rows M[row, b]
    rp = np.full((81, 9), -1.0 / 9.0, np.float32)
    for a in range(9):
        for k in range(9):
            rp[a * 9 + k, k] += 1.0

    # Final linear with ln_w/ln_b folded in:
    # out = W_out[:, :9] @ this + (W_out[:, 9:] * ln_w) @ (bil_c * rstd) + b'
    l1 = np.ascontiguousarray(W_out[:, 0:9].T)
    l2 = np.ascontiguousarray((W_out[:, 9:18] * ln_w[None, :]).T)
    bout = (b_out + W_out[:, 9:18] @ ln_b).reshape(9, 1).astype(np.float32)

    i9 = np.eye(9, dtype=np.float32)
    o99 = np.full((9, 1), 1.0, np.float32)
    o19 = np.ones((1, 9), np.float32)

    return {
        "ws": ws, "tm": tm, "e9": e9, "rp": rp,
        "l1": l1, "l2": l2, "bout": bout, "i9": i9,
        "eps": np.full((9, 1), LN_EPS, np.float32),
        "o99": o99, "o19": o19,
    }


def build_program(b_core=B_CORE, num_devices=N_CORES):
    import concourse.bass as bass  # noqa: F401
    import concourse.tile as tile
    from concourse import bacc, bass_isa, mybir

    f32 = mybir.dt.float32
    f32r = mybir.dt.float32r
    bf16 = mybir.dt.bfloat16
    nc = bacc.Bacc("TRN2", target_bir_lowering=False, debug=False,
                   num_devices=num_devices)

    feat_d = nc.dram_tensor("feat", [b_core, D], bf16 if HOST_BF16 else f32,
                            kind="ExternalInput")
    out_d = nc.dram_tensor("out", [b_core, 9], f32, kind="ExternalOutput")
    cshapes = {
        "ws": [128, 48, 41], "tm": [41, 81], "e9": [9, 81], "rp": [81, 9],
        "l1": [9, 9], "l2": [9, 9], "bout": [9, 1], "i9": [9, 9],
        "eps": [9, 1], "o99": [9, 1], "o19": [1, 9],
    }
    f32r_keys = {"tm", "e9", "rp", "l1", "l2", "o99", "o19"}

    def cdt(k):
        return bf16 if k == "ws" else (f32r if k in f32r_keys else f32)
    cd = {k: nc.dram_tensor(k, v, cdt(k), kind="ExternalInput")
          for k, v in cshapes.items()}

    nbuf = b_core // NB
    with tile.TileContext(nc) as tc:
        with tc.tile_pool(name="consts", bufs=1) as cp, \
             tc.tile_pool(name="tin", bufs=2) as tinp, \
             tc.tile_pool(name="tr", bufs=3) as trp, \
             tc.tile_pool(name="ysb", bufs=2) as ysbp, \
             tc.tile_pool(name="episb", bufs=2) as esbp, \
             tc.tile_pool(name="outsb", bufs=2) as outp, \
             tc.tile_pool(name="yps", bufs=1, space="PSUM") as yp, \
             tc.tile_pool(name="epips", bufs=3, space="PSUM") as epp:

            cs = {k: cp.tile(v, cdt(k), tag=k, name=k)
                  for k, v in cshapes.items()}
            for k in cshapes:
                # consts load on the ACT HWDGE queue so the SP queue can
                # start streaming feat immediately
                nc.scalar.dma_start(cs[k][:], cd[k].ap())

            def emit_load_transpose_proj(ib):
                # rows of this buffer, mapped so partition q=32g+c slot
                # u=4uh+ul holds row 256g+64ul+2c+uh (contiguous 6KB runs)
                # plain contiguous load: partition q holds rows 8q..8q+7
                rows = feat_d.ap()[ib * NB:(ib + 1) * NB, :]
                src = rows.rearrange("(q u) d -> q u d", q=128)
                t_in = tinp.tile([128, 8, D], bf16, tag="t_in",
                                 name=f"tin{ib}")
                for kc in range(NCHUNK):
                    if HOST_BF16:
                        nc.sync.dma_start(
                            t_in[:, :, kc * DC:(kc + 1) * DC],
                            src[:, :, kc * DC:(kc + 1) * DC])
                    else:
                        # SWDGE (gpsimd) DMA casts fp32 -> bf16 in-path
                        nc.gpsimd.dma_start(
                            t_in[:, :, kc * DC:(kc + 1) * DC],
                            src[:, :, kc * DC:(kc + 1) * DC])
                y_ps = [yp.tile([41, 256], f32, tag=f"y{g}",
                               name=f"y{g}_{ib}") for g in range(4)]
                for kc in range(NCHUNK):
                    # TR[32g+f, b, col] = feat[row, 32(kc*BJC+b)+f] where
                    # col = 128*(u&1) + 4c + (u>>1), row = 256g + 8c + u
                    # (u = row slot, c = partition-in-slab); this col order
                    # makes the final store come out in 72-byte runs.
                    trt = trp.tile([128, BJC, 256], bf16, tag="tr",
                                   name=f"tr{ib}_{kc}")
                    nc.vector.transpose(
                        trt[:].rearrange("p b (ulo c umid) -> p umid ulo b c",
                                         ulo=2, c=32, umid=4),
                        t_in[:, :, kc * DC:(kc + 1) * DC]
                        .rearrange("p (umid ulo) (b c) -> p umid ulo b c",
                                   umid=4, c=32))
                    for j in range(BJC):
                        bj = kc * BJC + j
                        for g in range(4):
                            nc.tensor.matmul(
                                y_ps[g][:],
                                cs["ws"][32 * g:32 * (g + 1), bj, :],
                                trt[32 * g:32 * (g + 1), j, :],
                                start=(bj == 0), stop=(bj == 47),
                                tile_position=(32 * g, 0),
                            )
                return y_ps

            def emit_epi(ib, y_ps):
                y_sb = ysbp.tile([41, 4, 256], f32r, tag="y_sb",
                                 name=f"ysb{ib}")
                for g in range(4):
                    nc.scalar.copy(y_sb[:, g, :], y_ps[g][:])
                y2 = y_sb[:].rearrange("p g n -> p (g n)")
                for h in range(2):
                    yh = y2[:, h * 512:(h + 1) * 512]
                    g_ps = epp.tile([81, 512], f32, tag="ep",
                                    name=f"g{ib}_{h}")
                    nc.tensor.matmul(g_ps[:], cs["tm"][32:41, :],
                                     yh[32:41, :], tile_position=(32, 0))
                    tb_ps = epp.tile([81, 512], f32, tag="ep",
                                     name=f"tb{ib}_{h}")
                    nc.tensor.matmul(tb_ps[:], cs["e9"][:], yh[0:9, :])
                    tb_sb = esbp.tile([81, 512], f32, tag="tb",
                                      name=f"tbs{ib}_{h}")
                    nc.scalar.copy(tb_sb[:], tb_ps[:])
                    m_sb = esbp.tile([81, 512], f32r, tag="m",
                                     name=f"m{ib}_{h}")
                    nc.vector.tensor_mul(m_sb[:], g_ps[:], tb_sb[:])
                    bil_ps = epp.tile([9, 512], f32, tag="ep",
                                      name=f"bil{ib}_{h}")
                    nc.tensor.matmul(bil_ps[:], cs["rp"][:], m_sb[:])
                    bil_sb = esbp.tile([9, 512], f32, tag="bil",
                                       name=f"bils{ib}_{h}")
                    nc.scalar.copy(bil_sb[:], bil_ps[:])
                    if VAR_PE:
                        sq_sb = esbp.tile([9, 512], f32r, tag="sq",
                                          name=f"sq{ib}_{h}")
                        nc.vector.tensor_mul(sq_sb[:], bil_sb[:], bil_sb[:])
                        var_ps = epp.tile([1, 512], f32, tag="ep",
                                          name=f"var{ib}_{h}")
                        nc.tensor.matmul(var_ps[:], cs["o99"][:], sq_sb[:])
                        lnv = esbp.tile([1, 512], f32, tag="lnv",
                                        name=f"lnv{ib}_{h}")
                        nc.scalar.activation(lnv[:], var_ps[:],
                                             mybir.ActivationFunctionType.Ln,
                                             bias=cs["eps"][0:1, 0:1],
                                             scale=1.0 / 9.0)
                        rstd = esbp.tile([1, 512], f32r, tag="rstd",
                                         name=f"rstd{ib}_{h}")
                        nc.scalar.activation(rstd[:], lnv[:],
                                             mybir.ActivationFunctionType.Exp,
                                             scale=-0.5)
                        rb_ps = epp.tile([9, 512], f32, tag="ep",
                                         name=f"rb{ib}_{h}")
                        nc.tensor.matmul(rb_ps[:], cs["o19"][:], rstd[:])
                        ln_sb = esbp.tile([9, 512], f32r, tag="lnsb",
                                          name=f"lns{ib}_{h}")
                        nc.vector.tensor_mul(ln_sb[:], rb_ps[:], bil_sb[:])
                    else:
                        sq_sb = esbp.tile([9, 512], f32, tag="sq",
                                          name=f"sq{ib}_{h}")
                        nc.gpsimd.tensor_mul(sq_sb[:], bil_sb[:], bil_sb[:])
                        # var*9, broadcast to 9 partitions in one gpsimd op
                        var_bc = esbp.tile([9, 512], f32, tag="var",
                                           name=f"var{ib}_{h}")
                        nc.gpsimd.partition_all_reduce(
                            var_bc[:], sq_sb[:], channels=9,
                            reduce_op=bass_isa.ReduceOp.add)
                        lnv = esbp.tile([9, 512], f32, tag="lnv",
                                        name=f"lnv{ib}_{h}")
                        nc.scalar.activation(lnv[:], var_bc[:],
                                             mybir.ActivationFunctionType.Ln,
                                             bias=cs["eps"][:, 0:1],
                                             scale=1.0 / 9.0)
                        rstd = esbp.tile([9, 512], f32, tag="rstd",
                                         name=f"rstd{ib}_{h}")
                        nc.scalar.activation(rstd[:], lnv[:],
                                             mybir.ActivationFunctionType.Exp,
                                             scale=-0.5)
                        ln_sb = esbp.tile([9, 512], f32r, tag="lnsb",
                                          name=f"lns{ib}_{h}")
                        nc.gpsimd.tensor_mul(ln_sb[:], bil_sb[:], rstd[:])
                    o_ps = epp.tile([9, 512], f32, tag="ep",
                                    name=f"o{ib}_{h}")
                    nc.tensor.matmul(o_ps[:], cs["l2"][:], ln_sb[:],
                                     start=True, stop=False)
                    nc.tensor.matmul(o_ps[:], cs["l1"][:], yh[0:9, :],
                                     start=False, stop=True)
                    osb = esbp.tile([9, 512], f32, tag="osb",
                                    name=f"osb{ib}_{h}")
                    nc.scalar.activation(osb[:], o_ps[:],
                                         mybir.ActivationFunctionType.Identity,
                                         bias=cs["bout"][:, 0:1])
                    # transpose to row-major: col 128*ss+p of osb is row
                    # 512h + 256*(ss>>1) + 2p + (ss&1) of this buffer
                    ot_ps = epp.tile([128, 2, 2, 9], f32, tag="ep",
                                     name=f"ot{ib}_{h}")
                    for ss in range(4):
                        nc.tensor.matmul(
                            ot_ps[:, ss >> 1, ss & 1, :],
                            osb[:, ss * 128:(ss + 1) * 128],
                            cs["i9"][:],
                            is_transpose=True,
                            start=(ss == 0), stop=(ss == 3),
                        )
                    out_sb = outp.tile([128, 2, 2, 9], f32, tag="out_sb",
                                       name=f"outsb{ib}_{h}")
                    nc.scalar.copy(out_sb[:], ot_ps[:])
                    base = ib * NB + h * 512
                    nc.scalar.dma_start(
                        out_d.ap()[base:base + 512, :]
                        .rearrange("(g2 p s) k -> p g2 s k", g2=2, p=128, s=2),
                        out_sb[:],
                    )

            # software pipeline: proj(ib) is emitted before epi(ib-1) so the
            # in-order PE queue always has dense matmul work between the
            # vector-latency-bound epilogue chains
            prev = None
            for ib in range(nbuf):
                y_ps = emit_load_transpose_proj(ib)
                if prev is not None:
                    emit_epi(*prev)
                prev = (ib, y_ps)
            emit_epi(*prev)
    nc.compile()
    return nc


_PROGRAM = None


def _get_program():
    global _PROGRAM
    if _PROGRAM is None:
        _PROGRAM = build_program()
    return _PROGRAM


def kernel(feat, W_int, W_stim, trans, ln_w, ln_b, W_out, b_out,
           trace=False, trace_kwargs=None):
    _ensure_axon_hooks()
    from concourse.bass_utils import run_bass_kernel_spmd

    feat = np.asarray(feat, np.float32)
    feat2 = feat.reshape(B_FULL, D)
    if HOST_BF16:
        import ml_dtypes
        feat2 = feat2.astype(ml_dtypes.bfloat16)
    consts = make_consts(W_int, W_stim, trans, ln_w, ln_b, W_out, b_out)
    nc = _get_program()
    in_maps = []
    for c in range(N_CORES):
        m = {"feat": np.ascontiguousarray(feat2[c * B_CORE:(c + 1) * B_CORE])}
        m.update(consts)
        in_maps.append(m)
    res = run_bass_kernel_spmd(nc, in_maps, list(range(N_CORES)), trace=trace)
    out = np.concatenate([res.results[c]["out"] for c in range(N_CORES)], axis=0)
    kernel.last_results = res
    return np.ascontiguousarray(out, dtype=np.float32)
